# revision 10
# baseline (speedup 1.0000x reference)
"""8-core TRN2 Bass kernel for nn_CFChurn_DNN (GNN message passing).

Self-contained: host-side graph partitioning + Bass/Tile kernel build +
SPMD execution on 8 NeuronCores via bass2jax/PJRT (axon).
See DESIGN notes in the original workspace for the derivation.
"""
import sys

sys.path.insert(0, "/opt/trn_rl_repo")

import numpy as np
import ml_dtypes

import concourse.bass as bass
import concourse.mybir as mybir
import concourse.tile as tile
import concourse.bacc as bacc

F32 = mybir.dt.float32
BF16 = mybir.dt.bfloat16
I16 = mybir.dt.int16
AF = mybir.ActivationFunctionType
ALU = mybir.AluOpType
AX = mybir.AxisListType

bf = ml_dtypes.bfloat16


class CFG:
    def __init__(self, N, E, NC=8, CE=4096):
        self.N, self.E, self.NC, self.CE = N, E, NC, CE
        self.NN = N // NC                      # real nodes per core
        self.NNp = ((self.NN + 127) // 128) * 128
        self.NPT = NC * self.NNp               # padded global gpos space
        assert self.NPT % 256 == 0
        self.HALFB = self.NPT // 2             # gpos < HALFB -> table A
        assert self.HALFB % 128 == 0 and (NC // 2) * self.NNp == self.HALFB
        self.RANKS = 1 + self.HALFB // 128     # incl zero rank 0
        self.NTOK = self.RANKS * 128
        assert self.NTOK < 32768
        self.H = 128


# ---------------------------------------------------------------- host prep

def _host_prep(cfg, edge_index):
    NC, NN, NNp = cfg.NC, cfg.NN, cfg.NNp
    src = np.asarray(edge_index[0], np.int64)
    dst = np.asarray(edge_index[1], np.int64)
    N = cfg.N
    deg = (np.bincount(dst, minlength=N) + 1.0).astype(np.float64)
    dinv = (deg ** -0.5).astype(np.float32)
    invdeg = (1.0 / deg).astype(np.float32)
    cnt = np.maximum(np.bincount(dst, minlength=N), 1.0)
    invcnt = (1.0 / cnt).astype(np.float32)

    src_isA = (src // NN) < (NC // 2)
    P0 = NNp - NN  # pad positions at front

    cores = []
    gpos = np.empty(N, np.int64)
    for c in range(NC):
        m = (dst // NN) == c
        es, ed = src[m], dst[m] - c * NN
        ea_rows = np.nonzero(m)[0]  # original edge ids for attr lookup
        isA = src_isA[m]
        dA = np.bincount(ed[isA], minlength=NN)
        dB = np.bincount(ed[~isA], minlength=NN)
        order = np.lexsort((dB, dA))           # local node ids by position-P0
        pos_of = np.empty(NN, np.int64)
        pos_of[order] = np.arange(NN) + P0
        gpos[c * NN:(c + 1) * NN] = c * NNp + pos_of
        # per local node: lists of (token, attr_row) per half
        cores.append(dict(es=es, ed=ed, eids=ea_rows, isA=isA, order=order,
                          dA=dA, dB=dB, pos_of=pos_of))

    seqA = np.zeros((NC, NNp), np.int64)
    seqB = np.zeros((NC, NNp), np.int64)
    for c in range(NC):
        o = cores[c]["order"]
        seqA[c, P0:] = cores[c]["dA"][o]
        seqB[c, P0:] = cores[c]["dB"][o]
    slotA = seqA.max(0)
    slotB = seqB.max(0)

    # chunk structure per pass: list of dicts(p0,p1,cols,n_idx,runs)
    def make_chunks(slot):
        chunks = []
        p = 0
        while p < NNp:
            w = 0
            p0 = p
            while p < NNp and (w + slot[p] <= cfg.CE or w == 0):
                w += int(slot[p]); p += 1
            n_idx = ((w + 127) // 128) * 128
            runs = []
            q = p0
            while q < p:
                d = int(slot[q])
                r = q
                while r < p and slot[r] == d:
                    r += 1
                if d > 0:
                    runs.append((d, q, r - q))
                q = r
            chunks.append(dict(p0=p0, p1=p, w=w, n_idx=max(n_idx, 128), runs=runs))
        return chunks

    chA, chB = make_chunks(slotA), make_chunks(slotB)

    # per-core idx arrays + ea arrays, aligned with chunk layout
    totA = sum(ch["n_idx"] for ch in chA)
    totB = sum(ch["n_idx"] for ch in chB)
    meta = dict(cfg=cfg, chA=chA, chB=chB, totA=totA, totB=totB,
                slotA=slotA, slotB=slotB)

    per_core = []
    for c in range(NC):
        co = cores[c]
        # group edges by (half, position): token + attr row
        tokA = [[] for _ in range(NNp)]
        tokB = [[] for _ in range(NNp)]
        atA = [[] for _ in range(NNp)]
        atB = [[] for _ in range(NNp)]
        posv = co["pos_of"][co["ed"]]
        tok_g = gpos[co["es"]]
        tokh = np.where(tok_g < cfg.HALFB, tok_g, tok_g - cfg.HALFB) + 128
        for i in range(len(co["es"])):
            p = posv[i]
            if co["isA"][i]:
                tokA[p].append(int(tokh[i])); atA[p].append(int(co["eids"][i]))
            else:
                tokB[p].append(int(tokh[i])); atB[p].append(int(co["eids"][i]))

        def build(chunks, slot, tok, att):
            idx_parts, at_parts, pad_parts = [], [], []
            for ch in chunks:
                ii, aa = [], []
                for p in range(ch["p0"], ch["p1"]):
                    d = int(slot[p])
                    lst, al = tok[p], att[p]
                    ii.extend(lst); aa.extend(al)
                    ii.extend([0] * (d - len(lst)))
                    aa.extend([-1] * (d - len(lst)))
                ii.extend([0] * (ch["n_idx"] - len(ii)))
                aa.extend([-1] * (ch["n_idx"] - len(aa)))
                idx_parts.append(np.array(ii, np.int16))
                at_parts.append(np.array(aa, np.int64))
            return np.concatenate(idx_parts), np.concatenate(at_parts)

        idxA, atA_arr = build(chA, slotA, tokA, atA)
        idxB, atB_arr = build(chB, slotB, tokB, atB)
        per_core.append(dict(idxA=idxA, idxB=idxB, atA=atA_arr, atB=atB_arr,
                             order=co["order"]))

    meta["per_core"] = per_core
    meta["gpos"] = gpos
    meta["dinv"] = dinv
    meta["invdeg"] = invdeg
    meta["invcnt"] = invcnt
    meta["P0"] = P0
    return meta


def _wrap_idx(a):
    # [L] int16 -> [128, L/16] with idx j at [j%16, j//16], tiled 8x on partitions
    return np.tile(a.reshape(-1, 16).T, (8, 1)).copy()


def _host_inputs(meta, discrete_x, continous_x, edge_attr, params, t):
    cfg = meta["cfg"]
    NC, NNp, P0, NN = cfg.NC, cfg.NNp, meta["P0"], cfg.NN
    p = {k: np.asarray(v, np.float32) for k, v in params.items()}

    Wcb = np.zeros((96, 96), np.float32)
    for g in range(3):
        Wcb[g * 32:(g + 1) * 32, g * 32:(g + 1) * 32] = p["W_c"]
    bc3 = np.tile(p["b_c"], 3)[:, None]

    def col(v):
        return np.asarray(v, np.float32).reshape(-1, 1)

    Wec = np.zeros((18, 128), np.float32)
    Wec[0:16] = p["el_W"][256:272]
    Wec[16] = p["el_b"]
    Wec[17] = -1e9

    def ktiles(W):  # [K,M] -> [128, nk*nm, 128]
        K, M = W.shape
        nk, nm = K // 128, M // 128
        out = np.zeros((128, nk * nm, 128), np.float32)
        for a in range(nk):
            for b_ in range(nm):
                out[:, a * nm + b_, :] = W[a * 128:(a + 1) * 128,
                                           b_ * 128:(b_ + 1) * 128]
        return out

    weights = dict(
        Wcb=Wcb, bc3=bc3, Wg0=p["W_g0"], bg0=col(p["b_g0"]),
        g1W=p["gcn1_W"], g1b=col(p["gcn1_b"]),
        g2W=p["gcn2_W"], g2b=col(p["gcn2_b"]),
        elWa=p["el_W"][0:128], elWb=p["el_W"][128:256], Wec=Wec,
        resW1=ktiles(p["res_W1"]), resb1=p["res_b1"].reshape(2, 128).T.copy(),
        resW2=ktiles(p["res_W2"]), resb2=p["res_b2"].reshape(2, 128).T.copy(),
        fusW=ktiles(p["fus_W"]).reshape(128, -1), fusb=col(p["fus_b"]),
        si0W=ktiles(p["si0_W"]).reshape(128, -1), si0b=col(p["si0_b"]),
        c1Wn=p["c1_Wn"], c1We=p["c1_We"], c1Wr=p["c1_Wr"], c1b=col(p["c1_b"]),
        c2Wn=p["c2_Wn"], c2We=p["c2_We"], c2Wr=p["c2_Wr"], c2b=col(p["c2_b"]),
        at0W=ktiles(p["attn0_W"]), at0b=p["attn0_b"].reshape(2, 128).T.copy(),
        at1W=ktiles(p["attn1_W"]), at1b=p["attn1_b"].reshape(2, 128).T.copy(),
        y0W1=p["y0_W1"], y0b1=col(p["y0_b1"]), y0W2=p["y0_W2"],
        y0b2=p["y0_b2"].reshape(1, 1),
        y1W1=p["y1_W1"], y1b1=col(p["y1_b1"]), y1W2=p["y1_W2"],
        y1b2=p["y1_b2"].reshape(1, 1),
        TW=p["T_W"], Tb=p["T_b"].reshape(1, 1),
    )

    xd = np.asarray(discrete_x, np.float32)[:, :32]
    xc = np.asarray(continous_x, np.float32)
    ea = np.asarray(edge_attr, np.float32)
    t_np = np.asarray(t, np.float32)
    dinv, invdeg, invcnt = meta["dinv"], meta["invdeg"], meta["invcnt"]

    in_maps = []
    for c in range(NC):
        pc = meta["per_core"][c]
        order = pc["order"]
        nodes = c * NN + order  # original node ids at positions P0..NNp-1
        xdc = np.zeros((128, NNp), bf)
        xdc[0:32, P0:] = xd[nodes].T.astype(bf)
        xdc[32:128, P0:] = xc[nodes].T.astype(bf)
        rows = np.zeros((3, NNp), bf)
        rows[0, P0:] = dinv[nodes].astype(bf)
        rows[1, P0:] = invdeg[nodes].astype(bf)
        rows[2, P0:] = invcnt[nodes].astype(bf)
        dinv_lb = np.ones((128, NNp // 128), np.float32)
        dl = np.ones(NNp, np.float32)
        dl[P0:] = dinv[nodes]
        dinv_lb[:, :] = dl.reshape(NNp // 128, 128).T

        def ea_cols(at):
            out = np.zeros((18, len(at)), bf)
            real = at >= 0
            out[0:16, real] = ea[at[real]].T.astype(bf)
            out[16, :] = bf(1.0)
            out[17, ~real] = bf(1.0)
            return out

        m = dict(
            xdc=xdc,
            rows=rows,
            dinv_lb=dinv_lb,
            idxA=_wrap_idx(pc["idxA"]), idxB=_wrap_idx(pc["idxB"]),
            eaA=ea_cols(pc["atA"]), eaB=ea_cols(pc["atB"]),
        )
        for k, v in weights.items():
            m[k] = v.astype(bf) if k in BF16_W else v
        in_maps.append(m)
    return in_maps


# ---------------------------------------------------------------- device build

BF16_W = ["Wcb", "Wg0", "g1W", "g2W", "elWa", "elWb", "Wec", "resW1",
          "resW2", "fusW", "si0W", "c1Wn", "c1Wr", "c2Wn", "c2Wr",
          "at0W", "at1W", "y0W1", "y0W2", "y1W1", "y1W2", "TW"]

WSHAPES = dict(Wcb=(96, 96), bc3=(96, 1), Wg0=(128, 128), bg0=(128, 1),
               g1W=(128, 128), g1b=(128, 1), g2W=(128, 128), g2b=(128, 1),
               elWa=(128, 128), elWb=(128, 128), Wec=(18, 128),
               resW1=(128, 4, 128), resb1=(128, 2),
               resW2=(128, 4, 128), resb2=(128, 2),
               fusW=(128, 256), fusb=(128, 1),
               si0W=(128, 256), si0b=(128, 1),
               c1Wn=(128, 128), c1We=(128, 128), c1Wr=(128, 128),
               c1b=(128, 1),
               c2Wn=(128, 128), c2We=(128, 128), c2Wr=(128, 128),
               c2b=(128, 1),
               at0W=(128, 4, 128), at0b=(128, 2),
               at1W=(128, 4, 128), at1b=(128, 2),
               y0W1=(128, 128), y0b1=(128, 1), y0W2=(128, 1), y0b2=(1, 1),
               y1W1=(128, 128), y1b1=(128, 1), y1W2=(128, 1), y1b2=(1, 1),
               TW=(128, 1), Tb=(1, 1))


def _build(meta):
    cfg = meta["cfg"]
    NC, NNp = cfg.NC, cfg.NNp
    n = NNp
    RANKS = cfg.RANKS

    nc = bacc.Bacc("TRN2", target_bir_lowering=False, debug=False,
                   num_devices=NC)

    def din(name, shape, dt=F32):
        return nc.dram_tensor(name, list(shape), dt, kind="ExternalInput")

    T_xdc = din("xdc", [128, n], BF16)
    T_rows = din("rows", [3, n], BF16)
    T_dlb = din("dinv_lb", [128, n // 128])
    T_idxA = din("idxA", [128, meta["totA"] // 16], I16)
    T_idxB = din("idxB", [128, meta["totB"] // 16], I16)
    T_eaA = din("eaA", [18, meta["totA"]], BF16)
    T_eaB = din("eaB", [18, meta["totB"]], BF16)
    WS = {k: din(k, list(sh), BF16 if k in BF16_W else F32)
          for k, sh in WSHAPES.items()}

    O = {}
    for k in ["predy0", "predy1", "predT"]:
        O[k] = nc.dram_tensor(k, [1, n], F32, kind="ExternalOutput")
    O["hci"] = nc.dram_tensor("hci", [128, n], F32, kind="ExternalOutput")
    O["hsi"] = nc.dram_tensor("hsi", [128, n], F32, kind="ExternalOutput")

    def NCH(total, s=512):
        return [(j, min(s, total - j)) for j in range(0, total, s)]

    with tile.TileContext(nc) as tc:
        with tc.tile_pool(name="w", bufs=1) as wp, \
             tc.tile_pool(name="nd", bufs=1) as np_, \
             tc.tile_pool(name="sc", bufs=2) as sc, \
             tc.tile_pool(name="ps", bufs=4, space="PSUM") as psp, \
             tc.tile_pool(name="dram", bufs=1, space="DRAM") as dp:

            W = {}
            for k, sh in WSHAPES.items():
                W[k] = wp.tile(list(sh), BF16 if k in BF16_W else F32,
                               tag=k, name="w_" + k)
                nc.sync.dma_start(W[k][:], WS[k][:])
            dlb = wp.tile([128, n // 128], F32, tag="dlb")
            nc.sync.dma_start(dlb[:], T_dlb[:])
            ones1 = wp.tile([1, 128], BF16, tag="ones1")
            nc.vector.memset(ones1[:], 1.0)
            ones128b = wp.tile([128, 1], BF16, tag="ones128b")
            nc.vector.memset(ones128b[:], 1.0)

            def b16s(tag):
                return np_.tile([128, n], BF16, tag=tag, name="b16_" + tag)



            def bcast_row(r, tag, pool=None):
                out = (pool or np_).tile([128, n], BF16, tag=tag,
                                         name="bc_" + tag)
                for j, w in NCH(n):
                    rb = sc.tile([1, 512], BF16, tag="rb", name="rb")
                    nc.sync.dma_start(rb[:, :w], T_rows[r:r + 1, j:j + w])
                    ps = psp.tile([128, 512], F32, tag="ps")
                    nc.tensor.matmul(ps[:, :w], ones1[:], rb[:, :w],
                                     start=True, stop=True)
                    nc.vector.tensor_copy(out[:, j:j + w], ps[:, :w])
                return out

            def gemm(wks, consumer):
                for j, w in NCH(n):
                    ps = psp.tile([128, 512], F32, tag="ps")
                    for ki, (wk, rh) in enumerate(wks):
                        nc.tensor.matmul(ps[:, :w], wk, rh[:, j:j + w],
                                         start=(ki == 0),
                                         stop=(ki == len(wks) - 1))
                    consumer(ps, j, w)

            def relu_out(dst, bias):
                def f(ps, j, w):
                    nc.vector.tensor_scalar(dst[:, j:j + w], ps[:, :w],
                                            bias, 0.0, ALU.add, ALU.max)
                return f

            # ---- stage 1
            xcr = np_.tile([96, n], BF16, tag="bs0", name="xcr")
            nc.sync.dma_start(xcr[:], T_xdc[32:128, :])
            xall = b16s("bs1")
            nc.sync.dma_start(xall[0:32, :], T_xdc[0:32, :])
            xc3 = np_.tile([96, n], BF16, tag="bs2", name="xc3")
            for j, w in NCH(n):
                ps = psp.tile([128, 512], F32, tag="ps")
                nc.tensor.matmul(ps[:96, :w], W["Wcb"][:], xcr[:, j:j + w],
                                 start=True, stop=True)
                nc.vector.tensor_scalar(xc3[:, j:j + w], ps[:96, :w],
                                        W["bc3"][:], 0.0, ALU.add, ALU.max)
            nc.sync.dma_start(xall[32:128, :], xc3[:])
            xg = b16s("bs2")
            gemm([(W["Wg0"][:], xall)], relu_out(xg, W["bg0"][:]))

            hci_sp = dp.tile([128, n], BF16)   # hcib spill

            # ================= graph phase (scoped pools) =================
            with tc.tile_pool(name="tab", bufs=1) as tp, \
                 tc.tile_pool(name="g", bufs=2) as gp, \
                 tc.tile_pool(name="gf", bufs=1) as gf, \
                 tc.tile_pool(name="tmpp", bufs=1) as tpp:

                tabt = tp.tile([128, RANKS * 128], BF16, tag="tab",
                               name="tabt")

                def f32s(tag):
                    return gf.tile([128, n], F32, tag="fS", name="f32_" + tag)

                def build_rows(x_slab, Wap, scale, rows_dram):
                    for blk in range(n // 128):
                        ps = psp.tile([128, 512], F32, tag="ps")
                        nc.tensor.matmul(ps[:, :128],
                                         x_slab[:, blk * 128:(blk + 1) * 128],
                                         Wap, start=True, stop=True)
                        sb = sc.tile([128, 512], BF16, tag="sb")
                        if scale is not None:
                            nc.vector.tensor_scalar_mul(sb[:, :128],
                                                        ps[:, :128],
                                                        scale[:, blk:blk + 1])
                        else:
                            nc.vector.tensor_copy(sb[:, :128], ps[:, :128])
                        nc.sync.dma_start(
                            rows_dram[blk * 128:(blk + 1) * 128, :],
                            sb[:, :128])

                def allgather(rows_dram):
                    ag = dp.tile([cfg.NPT, 128], BF16)
                    nc.gpsimd.collective_compute(
                        "AllGather", ALU.bypass,
                        replica_groups=[list(range(NC))],
                        ins=[rows_dram.opt()], outs=[ag.opt()])
                    return ag

                def spmm(ag, S, hook=None):
                    nc.vector.memset(S[:], 0.0)
                    for half, chunks, T_idx, T_ea in [
                            (1, meta["chB"], T_idxB, T_eaB),
                            (0, meta["chA"], T_idxA, T_eaA)]:
                        nc.vector.memset(tabt[:, 0:128], 0.0)
                        base = half * cfg.HALFB
                        nc.sync.dma_start(
                            tabt[:].rearrange("p (r f) -> p r f",
                                              f=128)[:, 1:RANKS, :],
                            ag[base:base + cfg.HALFB, :].rearrange(
                                "(r p) f -> p r f", p=128))
                        off = 0
                        for ch in chunks:
                            ni = ch["n_idx"]
                            if not ch["runs"]:
                                off += ni
                                continue
                            it = gp.tile([128, cfg.CE // 16 + 8], I16,
                                         tag="idx", name="it")
                            nc.sync.dma_start(
                                it[:, :ni // 16],
                                T_idx[:, off // 16:(off + ni) // 16])
                            g = gp.tile([128, 1, cfg.CE + 128], BF16,
                                        tag="g", name="g")
                            nc.gpsimd.dma_gather(
                                g[:, :, :ni], tabt[:], it[:, :ni // 16],
                                ni, ni, 128, transpose=True,
                                single_packet=False,
                                sbuf_tokens_per_rank=128,
                                sbuf_free_dim_per_rank=256)
                            if hook is not None:
                                hook(g, ch, T_ea, off)
                            if half == 1:
                                col = 0
                                for (d, pst, npos) in ch["runs"]:
                                    gv = g[:, 0,
                                           col:col + npos * d].rearrange(
                                        "p (m d) -> p m d", d=d)
                                    nc.vector.reduce_sum(
                                        S[:, pst:pst + npos], gv, axis=AX.X)
                                    col += npos * d
                            else:
                                tmp = tpp.tile([128, cfg.CE], F32,
                                               tag="tmp", name="tmp")
                                col = 0
                                r0 = ch["runs"][0][1]
                                r1 = ch["runs"][-1][1] + ch["runs"][-1][2]
                                for (d, pst, npos) in ch["runs"]:
                                    nc.vector.reduce_sum(
                                        tmp[:, pst - r0:pst - r0 + npos],
                                        g[:, 0,
                                          col:col + npos * d].rearrange(
                                            "p (m d) -> p m d", d=d),
                                        axis=AX.X)
                                    col += npos * d
                                nc.vector.tensor_add(S[:, r0:r1],
                                                     S[:, r0:r1],
                                                     tmp[:, 0:r1 - r0])
                            off += ni
                    return S

                Dinv_b = bcast_row(0, "bs3")
                InvDeg_b = bcast_row(1, "bs4")

                def gcn(x_in, Wkey, bias, out_tag):
                    rows_dram = dp.tile([n, 128], BF16)
                    build_rows(x_in, W[Wkey][:], dlb, rows_dram)
                    ag = allgather(rows_dram)
                    S = f32s("fA")
                    spmm(ag, S)
                    out = b16s(out_tag)

                    def cons(ps, j, w):
                        a1 = sc.tile([128, 512], F32, tag="s1")
                        nc.vector.tensor_mul(a1[:, :w], S[:, j:j + w],
                                             Dinv_b[:, j:j + w])
                        a2 = sc.tile([128, 512], F32, tag="s2")
                        nc.vector.tensor_mul(a2[:, :w], ps[:, :w],
                                             InvDeg_b[:, j:j + w])
                        nc.vector.tensor_add(a1[:, :w], a1[:, :w], a2[:, :w])
                        nc.vector.tensor_scalar(out[:, j:j + w], a1[:, :w],
                                                bias, 0.0, ALU.add, ALU.max)
                    gemm([(W[Wkey][:], x_in)], cons)
                    return out

                xg0 = gcn(xg, "g1W", W["g1b"][:], "bs5")
                xg1 = gcn(xg0, "g2W", W["g2b"][:], "bs0")
                xB = b16s("bs2")
                nc.vector.tensor_add(xB[:], xg0[:], xg1[:])
                xslabs = [xall, xB]

                # ---- ResDNN + fusion + x_si
                h1 = [b16s("bs5"), b16s("bs0")]
                for mt in range(2):
                    gemm([(W["resW1"][:, 0 * 2 + mt, :], xslabs[0]),
                          (W["resW1"][:, 1 * 2 + mt, :], xslabs[1])],
                         relu_out(h1[mt], W["resb1"][:, mt:mt + 1]))
                xdeep = [b16s("bs3"), b16s("bs4")]
                for mt in range(2):
                    def cons(ps, j, w, mt=mt):
                        a1 = sc.tile([128, 512], F32, tag="s1")
                        nc.vector.tensor_scalar_add(a1[:, :w], ps[:, :w],
                                                    W["resb2"][:, mt:mt + 1])
                        nc.vector.tensor_add(xdeep[mt][:, j:j + w],
                                             a1[:, :w],
                                             xslabs[mt][:, j:j + w])
                    gemm([(W["resW2"][:, 0 * 2 + mt, :], h1[0]),
                          (W["resW2"][:, 1 * 2 + mt, :], h1[1])], cons)

                def cons_hci(ps, j, w):
                    a1 = sc.tile([128, 512], F32, tag="s1")
                    nc.vector.tensor_scalar(a1[:, :w], ps[:, :w],
                                            W["fusb"][:], 0.0, ALU.add,
                                            ALU.max)
                    nc.sync.dma_start(O["hci"][:, j:j + w], a1[:, :w])
                    b1 = sc.tile([128, 512], BF16, tag="sb")
                    nc.vector.tensor_copy(b1[:, :w], a1[:, :w])
                    nc.sync.dma_start(hci_sp[:, j:j + w], b1[:, :w])
                gemm([(W["fusW"][:, 0:128], xdeep[0]),
                      (W["fusW"][:, 128:256], xdeep[1])], cons_hci)
                xsi = b16s("bs5")
                gemm([(W["si0W"][:, 0:128], xslabs[0]),
                      (W["si0W"][:, 128:256], xslabs[1])],
                     relu_out(xsi, W["si0b"][:]))

                # ---- e-stream
                bloc = b16s("bs0")
                gemm([(W["elWb"][:], xsi)],
                     lambda ps, j, w: nc.vector.tensor_copy(
                         bloc[:, j:j + w], ps[:, :w]))
                rows_a = dp.tile([n, 128], BF16)
                build_rows(xsi, W["elWa"][:], None, rows_a)
                ag_a = allgather(rows_a)
                rows_w = dp.tile([n, 128], BF16)
                build_rows(xsi, W["c1Wn"][:], None, rows_w)
                ag_w = allgather(rows_w)

                def e_hook(g, ch, T_ea, off):
                    ni = ch["n_idx"]
                    ea = gp.tile([18, cfg.CE + 128], BF16, tag="ea",
                                 name="ea")
                    nc.sync.dma_start(ea[:, :ni], T_ea[:, off:off + ni])
                    for s0, w in NCH(ni):
                        ps = psp.tile([128, 512], F32, tag="ps")
                        nc.tensor.matmul(ps[:, :w], W["Wec"][:],
                                         ea[:, s0:s0 + w], start=True,
                                         stop=True)
                        nc.vector.tensor_add(g[:, 0, s0:s0 + w],
                                             g[:, 0, s0:s0 + w], ps[:, :w])
                    col = 0
                    for (d, pst, npos) in ch["runs"]:
                        gv = g[:, 0, col:col + npos * d].rearrange(
                            "p (m d) -> p m d", d=d)
                        bv = bloc[:, pst:pst + npos].rearrange(
                            "p (m o) -> p m o", o=1).broadcast_to(
                            [128, npos, d])
                        nc.vector.tensor_add(gv, gv, bv)
                        col += npos * d
                    nc.vector.tensor_scalar_max(g[:, 0, :ni],
                                                g[:, 0, :ni], 0.0)

                Se = f32s("fA")
                spmm(ag_a, Se, hook=e_hook)
                Se_d = dp.tile([128, n], F32)
                nc.sync.dma_start(Se_d[:], Se[:])
                Sw = f32s("fB")
                spmm(ag_w, Sw)
                InvCnt_b = bcast_row(2, "bs3")

                def elconv(xn, Sw_, We, Wr, bias, outb):
                    t1 = Sw_
                    for j, w in NCH(n):
                        sech = sc.tile([128, 512], F32, tag="s2",
                                       name="sech")
                        nc.sync.dma_start(sech[:, :w], Se_d[:, j:j + w])
                        ps = psp.tile([128, 512], F32, tag="ps")
                        nc.tensor.matmul(ps[:, :w], W[We][:], sech[:, :w],
                                         start=True, stop=True)
                        nc.vector.tensor_add(t1[:, j:j + w], ps[:, :w],
                                             Sw_[:, j:j + w])
                    nc.vector.tensor_mul(t1[:], t1[:], InvCnt_b[:])

                    def cons(ps, j, w):
                        a1 = sc.tile([128, 512], F32, tag="s1")
                        nc.vector.tensor_add(a1[:, :w], ps[:, :w],
                                             t1[:, j:j + w])
                        nc.vector.tensor_scalar(outb[:, j:j + w], a1[:, :w],
                                                bias, 0.0, ALU.add, ALU.max)
                    gemm([(W[Wr][:], xn)], cons)

                xsi0 = b16s("bs1")
                elconv(xsi, Sw, "c1We", "c1Wr", W["c1b"][:], xsi0)
                rows_w2 = dp.tile([n, 128], BF16)
                build_rows(xsi0, W["c2Wn"][:], None, rows_w2)
                ag_w2 = allgather(rows_w2)
                Sw2 = f32s("fB")
                spmm(ag_w2, Sw2)
                xsi1 = b16s("bs2")
                elconv(xsi0, Sw2, "c2We", "c2Wr", W["c2b"][:], xsi1)

            # ================= heads =====================================
            hsib = b16s("bs4")
            for j, w in NCH(n):
                a1 = sc.tile([128, 512], F32, tag="s1")
                nc.vector.tensor_add(a1[:, :w], xsi0[:, j:j + w],
                                     xsi1[:, j:j + w])
                nc.sync.dma_start(O["hsi"][:, j:j + w], a1[:, :w])
                b1 = sc.tile([128, 512], BF16, tag="sb")
                nc.vector.tensor_copy(b1[:, :w], a1[:, :w])
                nc.vector.tensor_copy(hsib[:, j:j + w], b1[:, :w])

            with tc.tile_pool(name="hp", bufs=1) as hp:
                hcib = b16s("bs5")
                nc.sync.dma_start(hcib[:], hci_sp[:])

                for j, w in NCH(n):
                    ps = psp.tile([128, 512], F32, tag="ps")
                    nc.tensor.matmul(ps[:1, :w], W["TW"][:],
                                     hsib[:, j:j + w], start=True, stop=True)
                    a1 = sc.tile([1, 512], F32, tag="hrow", name="hrow")
                    nc.scalar.activation(a1[:, :w], ps[:1, :w], AF.Sigmoid,
                                         bias=W["Tb"][:])
                    nc.sync.dma_start(O["predT"][:, j:j + w], a1[:, :w])

                hslabs = [hcib, hsib]

                def attn(Wk, bk, py_tag, py_pool):
                    E = [b16s("bs0"), b16s("bs1")]
                    for mt in range(2):
                        def cons(ps, j, w, mt=mt):
                            nc.scalar.activation(E[mt][:, j:j + w],
                                                 ps[:, :w], AF.Exp,
                                                 bias=W[bk][:, mt:mt + 1])
                        gemm([(W[Wk][:, 0 * 2 + mt, :], hslabs[0]),
                              (W[Wk][:, 1 * 2 + mt, :], hslabs[1])], cons)
                    R = b16s("bs3")
                    for j, w in NCH(n):
                        ps = psp.tile([128, 512], F32, tag="ps")
                        nc.tensor.matmul(ps[:1, :w], ones128b[:],
                                         E[0][:, j:j + w], start=True,
                                         stop=False)
                        nc.tensor.matmul(ps[:1, :w], ones128b[:],
                                         E[1][:, j:j + w], start=False,
                                         stop=True)
                        cs = sc.tile([1, 512], F32, tag="hrow", name="cs")
                        nc.vector.reciprocal(cs[:, :w], ps[:1, :w])
                        csb = sc.tile([1, 512], BF16, tag="csb", name="csb")
                        nc.vector.tensor_copy(csb[:, :w], cs[:, :w])
                        ps2 = psp.tile([128, 512], F32, tag="ps2")
                        nc.tensor.matmul(ps2[:, :w], ones1[:], csb[:, :w],
                                         start=True, stop=True)
                        nc.vector.tensor_copy(R[:, j:j + w], ps2[:, :w])
                    py = py_pool.tile([128, n], BF16, tag=py_tag,
                                      name=py_tag)
                    for j, w in NCH(n):
                        a1 = sc.tile([128, 512], F32, tag="s1")
                        nc.vector.tensor_mul(a1[:, :w], E[0][:, j:j + w],
                                             hslabs[0][:, j:j + w])
                        a2 = sc.tile([128, 512], F32, tag="s2")
                        nc.vector.tensor_mul(a2[:, :w], E[1][:, j:j + w],
                                             hslabs[1][:, j:j + w])
                        nc.vector.tensor_add(a1[:, :w], a1[:, :w],
                                             a2[:, :w])
                        nc.vector.tensor_mul(py[:, j:j + w], a1[:, :w],
                                             R[:, j:j + w])
                    return py

                py0 = attn("at0W", "at0b", "bs2", np_)
                py1 = attn("at1W", "at1b", "py1", hp)

                def yhead(py, W1, b1, W2, b2, out):
                    z = hp.tile([128, n], BF16, tag="z", name="z")
                    gemm([(W[W1][:], py)], relu_out(z, W[b1][:]))
                    for j, w in NCH(n):
                        ps = psp.tile([128, 512], F32, tag="ps")
                        nc.tensor.matmul(ps[:1, :w], W[W2][:],
                                         z[:, j:j + w], start=True,
                                         stop=True)
                        a1 = sc.tile([1, 512], F32, tag="hrow", name="yh")
                        nc.scalar.activation(a1[:, :w], ps[:1, :w],
                                             AF.Sigmoid, bias=W[b2][:])
                        nc.sync.dma_start(out[:, j:j + w], a1[:, :w])

                yhead(py0, "y0W1", "y0b1", "y0W2", "y0b2", O["predy0"])
                yhead(py1, "y1W1", "y1b1", "y1W2", "y1b2", O["predy1"])

    nc.compile()
    return nc


# ---------------------------------------------------------------- runner

def _make_runner(nc, n_cores):
    import jax
    from jax.sharding import Mesh, PartitionSpec
    from jax.experimental.shard_map import shard_map
    from concourse.bass2jax import (_bass_exec_p, install_neuronx_cc_hook,
                                    partition_id_tensor)
    install_neuronx_cc_hook()
    partition_name = (nc.partition_id_tensor.name
                      if nc.partition_id_tensor else None)
    in_names, out_names, out_avals, zero_outs = [], [], [], []
    for alloc in nc.m.functions[0].allocations:
        if not isinstance(alloc, mybir.MemoryLocationSet):
            continue
        name = alloc.memorylocations[0].name
        if alloc.kind == "ExternalInput":
            if name != partition_name:
                in_names.append(name)
        elif alloc.kind == "ExternalOutput":
            out_names.append(name)
            shape = tuple(alloc.tensor_shape)
            dtype = mybir.dt.np(alloc.dtype)
            out_avals.append(jax.core.ShapedArray(shape, dtype))
            zero_outs.append(np.zeros(shape, dtype))
    n_params = len(in_names)
    all_in = list(in_names) + list(out_names)
    if partition_name is not None:
        all_in.append(partition_name)
    donate = tuple(range(n_params, n_params + len(out_avals)))

    def _body(*args):
        operands = list(args)
        if partition_name is not None:
            operands.append(partition_id_tensor())
        return tuple(_bass_exec_p.bind(
            *operands, out_avals=tuple(out_avals), in_names=tuple(all_in),
            out_names=tuple(out_names), lowering_input_output_aliases=(),
            sim_require_finite=False, sim_require_nnan=False, nc=nc))

    devices = jax.devices()[:n_cores]
    mesh = Mesh(np.asarray(devices), ("core",))
    sharded = jax.jit(
        shard_map(_body, mesh=mesh,
                  in_specs=(PartitionSpec("core"),) * (n_params + len(out_avals)),
                  out_specs=(PartitionSpec("core"),) * len(out_names),
                  check_rep=False),
        donate_argnums=donate, keep_unused=True)

    def run(in_maps):
        per_core = [[np.asarray(m[k]) for k in in_names] for m in in_maps]
        concat_in = [np.concatenate([per_core[c][i] for c in range(n_cores)],
                                    axis=0) for i in range(n_params)]
        cz = [np.zeros((n_cores * z.shape[0], *z.shape[1:]), z.dtype)
              for z in zero_outs]
        out = sharded(*concat_in, *cz)
        jax.block_until_ready(out)
        return [{name: np.asarray(out[i]).reshape(n_cores,
                                                  *out_avals[i].shape)[c]
                 for i, name in enumerate(out_names)}
                for c in range(n_cores)]

    return run


_CACHE = {}


def _get(cfg, edge_index):
    key = (cfg.N, cfg.E, edge_index[0][:64].tobytes(),
           edge_index[1][:64].tobytes())
    if key not in _CACHE:
        meta = _host_prep(cfg, edge_index)
        nc = _build(meta)
        run = _make_runner(nc, cfg.NC)
        _CACHE.clear()
        _CACHE[key] = (meta, run)
    return _CACHE[key]


def kernel(discrete_x, continous_x, edge_attr, params, edge_index, t):
    edge_index = np.asarray(edge_index)
    cfg = CFG(N=int(discrete_x.shape[0]), E=int(edge_index.shape[1]))
    meta, run = _get(cfg, edge_index)
    in_maps = _host_inputs(meta, discrete_x, continous_x, edge_attr, params, t)
    res = run(in_maps)

    N, NC, NN, NNp, P0 = cfg.N, cfg.NC, cfg.NN, cfg.NNp, meta["P0"]
    pred_y0 = np.zeros((N, 1), np.float32)
    pred_y1 = np.zeros((N, 1), np.float32)
    pred_T = np.zeros((N, 1), np.float32)
    h_ci = np.zeros((N, 128), np.float32)
    h_si = np.zeros((N, 128), np.float32)
    for c in range(NC):
        nodes = c * NN + meta["per_core"][c]["order"]
        r = res[c]
        pred_y0[nodes, 0] = r["predy0"][0, P0:]
        pred_y1[nodes, 0] = r["predy1"][0, P0:]
        pred_T[nodes, 0] = r["predT"][0, P0:]
        h_ci[nodes] = r["hci"][:, P0:].T
        h_si[nodes] = r["hsi"][:, P0:].T
    tf = np.asarray(t, np.float32)[:, None]
    pred_y = (1.0 - tf) * pred_y0 + tf * pred_y1
    pred_y_cf = tf * pred_y0 + (1.0 - tf) * pred_y1
    return (pred_y, pred_y_cf, pred_y0, pred_y1, pred_T, h_ci, h_si)
